# revision 28
# baseline (speedup 1.0000x reference)
"""Trainium2 Bass kernel for AttentionWithGeGLU pooling.

Math (per batch row b):
  v[s]   = mean_d x[b,s,d]^2 + eps
  rs[s]  = v^-1/2 = exp(-0.5*ln(v))
  t[s]   = sum_d x[b,s,d] * (ln_w*att_w)[d]
  scr    = rs * t            (att_b dropped: softmax is shift-invariant)
  c'[s]  = exp(scr - 0.5*ln(v)) = exp(scr)*rs
  praw[d]= sum_s c'[s] * x[s,d]      (unnormalized)
  pooled = praw * ln_w / sum_s exp(scr)   (host applies this)
  h      = pooled @ geglu_w + geglu_b;  out = val * gelu(gate)

Sharding: two NEFF launches, host bridge in between.
  A) data-parallel pooling over batch (4 batches/core), x shipped bf16
     in d-major layout [8, 128, NB, S] so the PE does the t matvec and
     the q (sum of squares) reduction, gpsimd+ACT do the squares, and
     the DVE only runs the pooled tensor_tensor_reduce accumulations.
     q's ones-matmul runs in fp8 DoubleRow mode (2 d-chunks per call).
     Score rows are exported; the softmax denominator is summed on host.
  B) tensor-parallel GeGLU: host normalizes + transposes pooled
     (tiny), each core computes its 512 val+gate columns in plain bf16
     (hi/lo compensated path kept as env fallback).
"""

import os
import numpy as np

B, S, D, OUT = 32, 2048, 1024, 4096
EPS = 1e-6
NCORES = 8
NB = B // NCORES          # batches per core
COLS = OUT // NCORES      # val columns per core
P = 128
NC = D // P               # d-chunks
SC = 512                  # seq chunk for pool phase A
NSC = S // SC
NT = S // P               # seq tiles per batch
GT = SC // P              # s-tiles per smaj unit

_cache = {}


def _build_nc_pool_smaj(sq_gp=4, tr_engines=("sync",)):
    """sq_gp: of 4 s-tiles per unit, how many squared on gpsimd (rest ACT).
    tr_engines: cycled per unit for issuing the transpose DMAs."""
    import concourse.bacc as bacc
    import concourse.mybir as mybir
    import concourse.tile as tile
    from contextlib import ExitStack

    f32 = mybir.dt.float32
    bf16 = mybir.dt.bfloat16
    fp8 = mybir.dt.float8e4
    AF = mybir.ActivationFunctionType
    OP = mybir.AluOpType
    AX = mybir.AxisListType

    nc = bacc.Bacc("TRN2", target_bir_lowering=False, debug=False,
                   enable_asserts=False, num_devices=NCORES)

    x_d = nc.dram_tensor("x", [NB, S, D], bf16, kind="ExternalInput").ap()
    a8_d = nc.dram_tensor("a8", [P, NC], bf16, kind="ExternalInput").ap()
    cst_d = nc.dram_tensor("cst", [1, 2], f32, kind="ExternalInput").ap()
    zob_d = nc.dram_tensor("zob", [2, P], bf16, kind="ExternalInput").ap()
    praw_d = nc.dram_tensor("praw", [NB, D], f32, kind="ExternalOutput").ap()
    scrw_d = nc.dram_tensor("scrw", [P, NT, NB], f32, kind="ExternalOutput").ap()

    with tile.TileContext(nc) as tc, ExitStack() as ctx:
        singles = ctx.enter_context(tc.tile_pool(name="singles", bufs=1))
        xpool = ctx.enter_context(tc.tile_pool(name="xp", bufs=9))
        xtpool = ctx.enter_context(tc.tile_pool(name="xtp", bufs=5))
        sqpool = ctx.enter_context(tc.tile_pool(name="sqp", bufs=3))
        rows = ctx.enter_context(tc.tile_pool(name="rows", bufs=5))
        bpool = ctx.enter_context(tc.tile_pool(name="bp", bufs=2))
        psum_t = ctx.enter_context(
            tc.tile_pool(name="pt", bufs=2, space="PSUM"))
        psum_tc = ctx.enter_context(
            tc.tile_pool(name="ptc", bufs=2, space="PSUM"))
        psum_pp = ctx.enter_context(
            tc.tile_pool(name="ppp", bufs=2, space="PSUM"))

        from concourse.hw_specs import get_activation_tables
        _tables = get_activation_tables(nc.m.arch)
        _set_id = list(_tables).index("natural_log_exp_and_others")
        _ld = mybir.InstLoadActFuncSet(
            name=nc.get_next_instruction_name(), ins=[], outs=[],
            act_func_set_id=_set_id,
        )
        nc.scalar.add_instruction(_ld)



        a8 = singles.tile([P, NC], bf16)
        nc.scalar.dma_start(out=a8, in_=a8_d)
        ident = singles.tile([1, 1], f32)
        nc.scalar.dma_start(out=ident, in_=cst_d[0:1, 0:1])
        eps_col = singles.tile([P, 1], f32)
        nc.scalar.dma_start(out=eps_col,
                            in_=cst_d[0:1, 1:2].to_broadcast([P, 1]))
        zrow = singles.tile([1, P], bf16)
        nc.scalar.dma_start(out=zrow, in_=zob_d[0:1, :])
        one_row = singles.tile([1, P], bf16)
        nc.scalar.dma_start(out=one_row, in_=zob_d[1:2, :])

        units = [(b, sc) for b in range(NB) for sc in range(NSC)]
        scr_map = {}
        pp_map = {}
        c4_map = {}
        q_map = {}
        xs_map = {}
        state = {}

        def front1(u):
            b, sc = u
            if sc == 0:
                scr_map[b] = bpool.tile([P, NT], f32, tag="scr_all",
                                        name="scr_all")
                pp_map[b] = psum_pp.tile([1, D], f32, tag="pp", name="pp")
            xs = xpool.tile([P, GT, D], bf16, tag="x")
            # load and transpose go through the SAME queue: in-queue
            # ordering makes the transpose race-free vs its input load
            # (cross-queue DMA->DMA ordering is broken on this runtime)
            q_eng = getattr(nc, tr_engines[(b * NSC + sc) % len(tr_engines)])
            q_eng.dma_start(
                out=xs,
                in_=x_d[b, sc * SC:(sc + 1) * SC, :].rearrange(
                    "(g p) d -> p g d", p=P))
            # blocked transpose: each [128,128] block transposed in place
            xT = xtpool.tile([P, GT, NC, P], bf16, tag="xT")
            q_eng.dma_start_transpose(
                xT.rearrange("p g c q -> p (g c) q"),
                xs.rearrange("p g (c q) -> p (g c) q", q=P))
            # t matvec: contraction over d, strided rhs across the g dim
            t_ps = psum_t.tile([1, SC], f32, tag="t")
            for c in range(NC):
                nc.tensor.matmul(t_ps.rearrange("o (g q) -> o g q", q=P),
                                 lhsT=a8[:, c:c + 1], rhs=xT[:, :, c, :],
                                 start=(c == 0), stop=(c == NC - 1))
            state[u] = [xs, t_ps]

        def front2(u):
            b, sc = u
            xs, t_ps = state[u]
            if sc == 0:
                q_map[b] = rows.tile([P, NT], f32, tag="qall", name="q_all")
            q_all = q_map[b]
            sq = sqpool.tile([P, GT, D], fp8, tag="sq")
            for g in range(GT):
                if g < sq_gp:
                    nc.gpsimd.tensor_mul(sq[:, g, :], xs[:, g, :], xs[:, g, :])
                else:
                    nc.scalar.activation(out=sq[:, g, :], in_=xs[:, g, :],
                                         func=AF.Square)
                nc.vector.reduce_sum(q_all[:, sc * GT + g:sc * GT + g + 1],
                                     sq[:, g, :], axis=AX.X)

        def back(u):
            b, sc = u
            xs, t_ps = state[u]
            if sc == 0:
                xs_map[b] = []
            xs_map[b].append(xs)
            scr_all = scr_map[b]
            q_all = q_map[b]
            # t row -> columns: per-unit PSUM tile; the zero matmul and the
            # 4 accumulating transposes are CONSECUTIVE in the PE stream
            # (interleaving an accumulation group with other matmuls
            # corrupts PSUM on HW)
            t_row = rows.tile([1, SC], f32, tag="trow")
            nc.scalar.activation(out=t_row, in_=t_ps, func=AF.Identity)
            tcol = psum_tc.tile([P, GT], f32, tag="tc")
            nc.tensor.matmul(tcol, lhsT=one_row, rhs=zrow[0:1, 0:GT],
                             start=True, stop=False)
            for g in range(GT):
                nc.tensor.matmul(
                    tcol[:, g:g + 1],
                    lhsT=t_row[0:1, g * P:(g + 1) * P], rhs=ident,
                    is_transpose=True, start=False, stop=(g == GT - 1))
            # smalls per unit on [128, 4] columns
            q4 = q_all[:, sc * GT:(sc + 1) * GT]
            lv = rows.tile([P, GT], f32, tag="lv")
            nc.scalar.activation(out=lv, in_=q4, func=AF.Ln,
                                 scale=1.0 / D, bias=eps_col)
            rs = rows.tile([P, GT], f32, tag="rs")
            nc.scalar.activation(out=rs, in_=lv, func=AF.Exp, scale=-0.5)
            scr = scr_all[:, sc * GT:(sc + 1) * GT]
            nc.vector.tensor_mul(scr, tcol, rs)
            s2 = rows.tile([P, GT], f32, tag="s2")
            nc.vector.scalar_tensor_tensor(
                out=s2, in0=lv, scalar=-0.5, in1=scr,
                op0=OP.mult, op1=OP.add)
            c4 = c4_map.setdefault(
                b, rows.tile([P, NT], bf16, tag="c16", name="c16"))
            nc.scalar.activation(out=c4[:, sc * GT:(sc + 1) * GT], in_=s2,
                                 func=AF.Exp)
            state.pop(u)
            if sc == NSC - 1:
                batch_end(b)

        def batch_end(b):
            scr_all = scr_map[b]
            pp = pp_map[b]
            q_map.pop(b)
            c16 = c4_map.pop(b)
            # pooled: all 32 matmuls consecutive in the PE stream
            # (two 512-col halves: a [1,1024] f32 matmul would cross banks)
            for j in range(NT):
                xs = xs_map[b][j // GT]
                g = j % GT
                for h in range(2):
                    nc.tensor.matmul(
                        pp[0:1, h * 512:(h + 1) * 512],
                        lhsT=c16[:, j:j + 1],
                        rhs=xs[:, g, h * 512:(h + 1) * 512],
                        start=(j == 0), stop=(j == NT - 1))
            xs_map.pop(b)
            nc.sync.dma_start(out=scrw_d[:, :, b], in_=scr_all)
            praw_row = rows.tile([1, D], f32, tag="praw")
            nc.scalar.activation(out=praw_row, in_=pp, func=AF.Identity)
            nc.sync.dma_start(out=praw_d[b:b + 1, :], in_=praw_row)

        front1(units[0])
        front2(units[0])
        for i in range(1, len(units)):
            front1(units[i])
            back(units[i - 1])
            front2(units[i])
        back(units[-1])

    nc.compile()
    return nc




def _pool_smaj_in_maps(x, ln_w, att_w):
    import ml_dtypes
    import jax
    import jax.numpy as jnp
    a = (ln_w * att_w[:, 0]).astype(np.float32)
    a8 = np.ascontiguousarray(
        a.reshape(NC, P).T).astype(ml_dtypes.bfloat16)
    cst = np.array([[1.0, EPS]], dtype=np.float32)
    zob = np.zeros((2, P), dtype=ml_dtypes.bfloat16)
    zob[1, :] = 1.0
    cpu = jax.devices("cpu")[0]
    with jax.default_device(cpu):
        xb = np.asarray(jax.device_get(jnp.asarray(x).astype(jnp.bfloat16)))
    return [
        {"x": xb[r * NB:(r + 1) * NB], "a8": a8, "cst": cst, "zob": zob}
        for r in range(NCORES)
    ]


def _pool_smaj_unshard(results, ln_w):
    pooled = np.empty((B, D), dtype=np.float32)
    for r in range(NCORES):
        praw = results[r]["praw"]          # [NB, D]
        scrw = results[r]["scrw"]          # [P, NT, NB]
        dsum = np.exp(scrw).sum(axis=(0, 1))   # [NB]
        pooled[r * NB:(r + 1) * NB] = praw * ln_w[None, :] / dsum[:, None]
    return pooled


def _build_nc_pool_dmaj(sq_gp_frac=7):
    """d-major pooling NEFF. See module docstring for the engine plan."""
    import concourse.bacc as bacc
    import concourse.mybir as mybir
    import concourse.tile as tile
    from contextlib import ExitStack

    f32 = mybir.dt.float32
    bf16 = mybir.dt.bfloat16
    fp8 = mybir.dt.float8e4
    AF = mybir.ActivationFunctionType
    OP = mybir.AluOpType
    PM = mybir.MatmulPerfMode

    nc = bacc.Bacc("TRN2", target_bir_lowering=False, debug=False,
                   enable_asserts=False, num_devices=NCORES)

    xT_d = nc.dram_tensor("xT", [NC, P, NB, S], bf16, kind="ExternalInput").ap()
    a8_d = nc.dram_tensor("a8", [P, NC], bf16, kind="ExternalInput").ap()
    cst_d = nc.dram_tensor("cst", [1, 2], f32, kind="ExternalInput").ap()
    one8_d = nc.dram_tensor("one8", [1, P], fp8, kind="ExternalInput").ap()
    praw_d = nc.dram_tensor("praw", [P, NC, NB], f32, kind="ExternalOutput").ap()
    scrw_d = nc.dram_tensor("scrw", [1, NB, S], f32, kind="ExternalOutput").ap()

    with tile.TileContext(nc) as tc, ExitStack() as ctx:
        singles = ctx.enter_context(tc.tile_pool(name="singles", bufs=1))
        xpool = ctx.enter_context(tc.tile_pool(name="xp", bufs=5))
        sqpool = ctx.enter_context(tc.tile_pool(name="sqp", bufs=3))
        rows = ctx.enter_context(tc.tile_pool(name="rows", bufs=4))
        psum_tq = ctx.enter_context(
            tc.tile_pool(name="ptq", bufs=3, space="PSUM"))

        # Preload the act table set with Ln+Exp+Square so ACT never swaps.
        from concourse.hw_specs import get_activation_tables
        _tables = get_activation_tables(nc.m.arch)
        _set_id = list(_tables).index("natural_log_exp_and_others")
        _ld = mybir.InstLoadActFuncSet(
            name=nc.get_next_instruction_name(), ins=[], outs=[],
            act_func_set_id=_set_id,
        )
        nc.scalar.add_instruction(_ld)

        # singles go through other engines' DMA queues so the first x
        # transfer isn't queued behind them on SP
        a8 = singles.tile([P, NC], bf16)
        nc.scalar.dma_start(out=a8, in_=a8_d)
        ones8 = singles.tile([P, 1], fp8)
        nc.scalar.dma_start(out=ones8,
                            in_=one8_d[0:1, 0:1].to_broadcast([P, 1]))
        eps_t = singles.tile([1, 1], f32)
        nc.scalar.dma_start(out=eps_t, in_=cst_d[0:1, 1:2])

        praw = singles.tile([P, NC, NB], f32)

        units = [(b, sc) for b in range(NB) for sc in range(NSC)]
        scr_rows_map = {}
        state = {}

        def front1(u):
            b, sc = u
            if sc == 0:
                scr_rows_map[b] = rows.tile([1, S], f32, tag="e",
                                            name="scr_rows")
            xt = xpool.tile([P, NC, SC], bf16, tag="x")
            src = xT_d[:, :, b, sc * SC:(sc + 1) * SC].rearrange(
                "c p s -> p c s")
            if u == (0, 0):
                # finer DMA granularity so compute starts sooner at fill
                for c in range(NC):
                    nc.sync.dma_start(out=xt[:, c, :], in_=src[:, c, :])
            else:
                nc.sync.dma_start(out=xt, in_=src)

            tq = psum_tq.tile([33, SC], f32, tag="tq")
            t_ps = tq[0:1, :]
            q_ps = tq[32:33, :]
            for c in range(NC):
                nc.tensor.matmul(t_ps, lhsT=a8[:, c:c + 1], rhs=xt[:, c, :],
                                 start=(c == 0), stop=(c == NC - 1))
            state[u] = [xt, tq, q_ps]

        def front2(u):
            xt, tq, q_ps = state[u]
            sq = sqpool.tile([P, NC, SC], fp8, tag="sq")
            for c in range(NC):
                if c < sq_gp_frac:
                    nc.gpsimd.tensor_mul(sq[:, c, :], xt[:, c, :], xt[:, c, :])
                else:
                    nc.scalar.activation(out=sq[:, c, :], in_=xt[:, c, :],
                                         func=AF.Square)
            # q: plain fp8 ones-matmuls (DoubleRow crashes walrus codegen)
            for c in range(NC):
                nc.tensor.matmul(
                    q_ps, lhsT=ones8, rhs=sq[:, c, :],
                    start=(c == 0), stop=(c == NC - 1))

        def rows_stage(u):
            b, sc = u
            xt, tq, q_ps = state[u]
            scr_rows = scr_rows_map[b]
            # one ACT copy (partitions 0 and 32 only) frees the PSUM bank
            # and gives gpsimd (which cannot read PSUM) an SBUF source for t
            t_sb = rows.tile([1, SC], f32, tag="tqs")
            nc.scalar.activation(out=t_sb, in_=tq[0:1, :], func=AF.Identity)
            lv = rows.tile([1, SC], f32, tag="lv")
            nc.scalar.activation(out=lv, in_=tq[32:33, :], func=AF.Ln,
                                 scale=1.0 / D, bias=eps_t)
            rs = rows.tile([1, SC], f32, tag="rs")
            nc.scalar.activation(out=rs, in_=lv, func=AF.Exp, scale=-0.5)
            scr = scr_rows[0:1, sc * SC:(sc + 1) * SC]
            nc.gpsimd.tensor_mul(scr, t_sb, rs)
            e_row = rows.tile([1, SC], f32, tag="erow")
            nc.scalar.activation(out=e_row, in_=scr, func=AF.Exp)
            c_row = rows.tile([1, SC], bf16, tag="c")
            nc.gpsimd.tensor_mul(c_row, e_row, rs)
            cbc = sqpool.tile([P, SC], bf16, tag="cbc")
            nc.gpsimd.partition_broadcast(cbc, c_row)
            state[u].append(cbc)
            if sc == NSC - 1:
                nc.sync.dma_start(out=scrw_d[0:1, b, :], in_=scr_rows)

        def ttr_stage(u):
            b, sc = u
            xt, tq, q_ps, cbc = state.pop(u)
            # pooled accumulation: free-axis reduce over s -> DVE only
            scrat = sqpool.tile([P, SC], bf16, tag="scrat")
            for c in range(NC):
                seed = 0.0 if sc == 0 else praw[:, c, b:b + 1]
                nc.vector.tensor_tensor_reduce(
                    out=scrat, in0=xt[:, c, :], in1=cbc,
                    scale=1.0, scalar=seed,
                    op0=OP.mult, op1=OP.add,
                    accum_out=praw[:, c, b:b + 1])
            if sc == NSC - 1:
                nc.sync.dma_start(out=praw_d[:, :, b:b + 1],
                                  in_=praw[:, :, b:b + 1])

        # software pipeline, stages skewed so the serial rows chain of
        # unit u resolves a full unit before its TTRs run
        front1(units[0])
        front2(units[0])
        for i in range(1, len(units)):
            front1(units[i])
            rows_stage(units[i - 1])
            if i >= 2:
                ttr_stage(units[i - 2])
            front2(units[i])
        rows_stage(units[-1])
        ttr_stage(units[-2])
        ttr_stage(units[-1])

    nc.compile()
    return nc


def _pool_dmaj_in_maps(x, ln_w, att_w):
    import ml_dtypes
    import jax
    import jax.numpy as jnp

    a = (ln_w * att_w[:, 0]).astype(np.float32)
    a8 = np.ascontiguousarray(
        a.reshape(NC, P).T).astype(ml_dtypes.bfloat16)      # [128, 8]
    cst = np.array([[1.0, EPS]], dtype=np.float32)
    one8 = np.ones((1, P), dtype=ml_dtypes.float8_e4m3)

    # bf16 conversion + d-major transpose on the (multithreaded) jax CPU
    # backend: [B, S, D] -> [NC, P, B, S]
    cpu = jax.devices("cpu")[0]
    with jax.default_device(cpu):
        xT = np.asarray(jax.device_get(
            jnp.asarray(x).astype(jnp.bfloat16)
            .reshape(B, S, NC, P).transpose(2, 3, 0, 1)))
    maps = []
    for r in range(NCORES):
        xTr = np.ascontiguousarray(xT[:, :, r * NB:(r + 1) * NB, :])
        maps.append({"xT": xTr, "a8": a8, "cst": cst, "one8": one8})
    return maps


def _pool_dmaj_unshard(results, ln_w):
    pooled = np.empty((B, D), dtype=np.float32)
    for r in range(NCORES):
        praw = results[r]["praw"]          # [128, 8, NB]
        scrw = results[r]["scrw"][0]       # [NB, S]
        dsum = np.exp(scrw).sum(axis=1)    # [NB]
        pr = praw.transpose(2, 1, 0).reshape(NB, D)
        pooled[r * NB:(r + 1) * NB] = pr * ln_w[None, :] / dsum[:, None]
    return pooled


def _build_nc_geglu_bf16(with_bias):
    """Plain-bf16 GeGLU readout: pooled^T [P, 8, B] bf16, per-core weight
    slice [8, P, 2*COLS] bf16. When the bias is all-zero (checked on the
    host), gelu reads straight from PSUM and the adds are skipped."""
    import concourse.bacc as bacc
    import concourse.mybir as mybir
    import concourse.tile as tile
    from contextlib import ExitStack

    f32 = mybir.dt.float32
    bf16 = mybir.dt.bfloat16
    AF = mybir.ActivationFunctionType

    nc = bacc.Bacc("TRN2", target_bir_lowering=False, debug=False,
                   enable_asserts=False, num_devices=NCORES)

    pT_d = nc.dram_tensor("pT", [P, NC, B], bf16, kind="ExternalInput").ap()
    w_d = nc.dram_tensor("w", [NC, P, 2 * COLS], bf16, kind="ExternalInput").ap()
    if with_bias:
        bias_d = nc.dram_tensor("bias", [1, 2 * COLS], f32,
                                kind="ExternalInput").ap()
    out_d = nc.dram_tensor("out", [B, COLS], f32, kind="ExternalOutput").ap()

    with tile.TileContext(nc) as tc, ExitStack() as ctx:
        singles = ctx.enter_context(tc.tile_pool(name="singles", bufs=1))
        tailp = ctx.enter_context(tc.tile_pool(name="tail", bufs=2))
        psum_pool = ctx.enter_context(
            tc.tile_pool(name="pspool", bufs=1, space="PSUM"))

        from concourse.hw_specs import get_activation_tables
        _tables = get_activation_tables(nc.m.arch)
        _set_id = list(_tables).index("gelu_and_others")
        _ld = mybir.InstLoadActFuncSet(
            name=nc.get_next_instruction_name(), ins=[], outs=[],
            act_func_set_id=_set_id,
        )
        nc.scalar.add_instruction(_ld)

        pT_sb = singles.tile([P, NC, B], bf16)
        nc.scalar.dma_start(out=pT_sb, in_=pT_d)
        # per-chunk DMAs so matmul k starts as soon as chunk k lands
        w_sb = singles.tile([P, NC, 2 * COLS], bf16)
        for k in range(NC):
            nc.sync.dma_start(out=w_sb[:, k], in_=w_d[k])
        if with_bias:
            bias_bc = singles.tile([B, 2 * COLS], f32)
            nc.sync.dma_start(out=bias_bc,
                              in_=bias_d.to_broadcast([B, 2 * COLS]))

        hps = psum_pool.tile([B, 2 * COLS], f32, tag="acc")
        for k in range(NC):
            for h in range(2):
                nc.tensor.matmul(
                    hps[:, h * COLS:(h + 1) * COLS],
                    lhsT=pT_sb[:, k, :],
                    rhs=w_sb[:, k, h * COLS:(h + 1) * COLS],
                    start=(k == 0), stop=(k == NC - 1),
                )
        if with_bias:
            import concourse.mybir as mb
            hv = tailp.tile([B, COLS], f32, tag="hv")
            nc.vector.tensor_add(hv, hps[:, 0:COLS], bias_bc[:, 0:COLS])
            hg = tailp.tile([B, COLS], f32, tag="hg")
            nc.vector.tensor_add(hg, hps[:, COLS:2 * COLS],
                                 bias_bc[:, COLS:2 * COLS])
            gg = tailp.tile([B, COLS], f32, tag="gg")
            nc.scalar.activation(out=gg, in_=hg, func=AF.Gelu)
            outt = tailp.tile([B, COLS], f32, tag="outt")
            nc.vector.tensor_mul(outt, hv, gg)
        else:
            gg = tailp.tile([B, COLS], f32, tag="gg")
            nc.scalar.activation(out=gg, in_=hps[:, COLS:2 * COLS],
                                 func=AF.Gelu)
            outt = tailp.tile([B, COLS], f32, tag="outt")
            nc.vector.tensor_mul(outt, hps[:, 0:COLS], gg)
        nc.sync.dma_start(out=out_d, in_=outt)

    nc.compile()
    return nc


_w_conv_cache = {}


def _geglu_bf16_in_maps(pooled_full, geglu_w, geglu_b, with_bias):
    import ml_dtypes

    pT = np.ascontiguousarray(
        pooled_full.T.astype(ml_dtypes.bfloat16).reshape(NC, P, B)
        .transpose(1, 0, 2))                                # [P, 8, B]

    key = (id(geglu_w), with_bias)
    cached = _w_conv_cache.get(key)
    if cached is None:
        wrs, brs = [], []
        for r in range(NCORES):
            vs = slice(r * COLS, (r + 1) * COLS)
            gs = slice(OUT + r * COLS, OUT + (r + 1) * COLS)
            wcat = np.concatenate([geglu_w[:, vs], geglu_w[:, gs]], axis=1)
            wrs.append(np.ascontiguousarray(
                wcat.astype(ml_dtypes.bfloat16).reshape(NC, P, 2 * COLS)))
            brs.append(np.ascontiguousarray(
                np.concatenate([geglu_b[vs], geglu_b[gs]])
            ).reshape(1, 2 * COLS).astype(np.float32))
        cached = (wrs, brs)
        _w_conv_cache[key] = cached
    wrs, brs = cached
    maps = []
    for r in range(NCORES):
        m = {"pT": pT, "w": wrs[r]}
        if with_bias:
            m["bias"] = brs[r]
        maps.append(m)
    return maps


# ---------------------------------------------------------------------------
# Classic (fallback) builders, kept from the validated baseline.

def _build_nc_pool_classic():
    """Conservative pool NEFF: fp32 x, per-tile DMAs, per-batch softmax,
    fp32 matmuls — mirrors the structure already proven to execute on HW."""
    import concourse.bacc as bacc
    import concourse.mybir as mybir
    import concourse.tile as tile
    from contextlib import ExitStack

    f32 = mybir.dt.float32
    AF = mybir.ActivationFunctionType
    OP = mybir.AluOpType
    AX = mybir.AxisListType

    nc = bacc.Bacc("TRN2", target_bir_lowering=False, debug=False,
                   enable_asserts=False, num_devices=NCORES)

    x_d = nc.dram_tensor("x", [NB, S, D], f32, kind="ExternalInput").ap()
    a_d = nc.dram_tensor("a", [1, D], f32, kind="ExternalInput").ap()
    lnw_d = nc.dram_tensor("lnw", [1, D], f32, kind="ExternalInput").ap()
    cst_d = nc.dram_tensor("cst", [1, 2], f32, kind="ExternalInput").ap()
    pooled_d = nc.dram_tensor("pooled", [NB, D], f32, kind="ExternalOutput").ap()

    with tile.TileContext(nc) as tc, ExitStack() as ctx:
        singles = ctx.enter_context(tc.tile_pool(name="singles", bufs=1))
        xpool = ctx.enter_context(tc.tile_pool(name="xp", bufs=26))
        scratch = ctx.enter_context(tc.tile_pool(name="scr", bufs=2))
        small = ctx.enter_context(tc.tile_pool(name="small", bufs=3))
        psum_pool = ctx.enter_context(tc.tile_pool(name="pspool", bufs=2, space="PSUM"))
        psum_small = ctx.enter_context(tc.tile_pool(name="pssm", bufs=2, space="PSUM"))

        a_bc = singles.tile([P, D], f32)
        nc.sync.dma_start(out=a_bc, in_=a_d.to_broadcast([P, D]))
        lnw_sb = singles.tile([1, D], f32)
        nc.sync.dma_start(out=lnw_sb, in_=lnw_d)
        ones = singles.tile([P, 1], f32)
        nc.sync.dma_start(out=ones, in_=cst_d[0:1, 0:1].to_broadcast([P, 1]))
        eps_col = singles.tile([P, 1], f32)
        nc.sync.dma_start(out=eps_col, in_=cst_d[0:1, 1:2].to_broadcast([P, 1]))

        pooled_sb = singles.tile([1, NB, D], f32)

        for b in range(NB):
            q_all = small.tile([P, NT], f32, tag="q")
            t_all = small.tile([P, NT], f32, tag="t")
            x_tiles = []
            for j in range(NT):
                xt = xpool.tile([P, D], f32, tag="x")
                nc.sync.dma_start(out=xt, in_=x_d[b, j * P:(j + 1) * P, :])
                x_tiles.append(xt)
                sq = scratch.tile([P, D], f32, tag="sq")
                nc.scalar.activation(out=sq, in_=xt, func=AF.Square)
                nc.vector.reduce_sum(q_all[:, j:j + 1], sq, axis=AX.X)
                tp = scratch.tile([P, D], f32, tag="tp")
                nc.vector.tensor_mul(tp, xt, a_bc)
                nc.vector.reduce_sum(t_all[:, j:j + 1], tp, axis=AX.X)

            rs = small.tile([P, NT], f32, tag="rs")
            nc.scalar.activation(out=rs, in_=q_all, func=AF.Sqrt,
                                 scale=1.0 / D, bias=eps_col)
            nc.vector.reciprocal(rs, rs)
            sc = small.tile([P, NT], f32, tag="sc")
            nc.vector.tensor_mul(sc, t_all, rs)
            e_all = small.tile([P, NT], f32, tag="e")
            nc.scalar.activation(out=e_all, in_=sc, func=AF.Exp)
            c_all = small.tile([P, NT], f32, tag="c")
            nc.vector.tensor_mul(c_all, e_all, rs)

            dps = psum_small.tile([1, NT], f32, tag="sm")
            nc.tensor.matmul(dps, lhsT=ones, rhs=e_all, start=True, stop=True)
            dsum = small.tile([1, 1], f32, tag="dsum")
            nc.vector.reduce_sum(dsum, dps, axis=AX.X)
            invd = small.tile([1, 1], f32, tag="invd")
            nc.vector.reciprocal(invd, dsum)

            pp = psum_pool.tile([1, D], f32, tag="acc")
            for j in range(NT):
                for h in range(2):
                    nc.tensor.matmul(
                        pp[0:1, h * 512:(h + 1) * 512],
                        lhsT=c_all[:, j:j + 1],
                        rhs=x_tiles[j][:, h * 512:(h + 1) * 512],
                        start=(j == 0), stop=(j == NT - 1))
            nc.vector.scalar_tensor_tensor(
                out=pooled_sb[0:1, b, :], in0=pp[0:1, :], scalar=invd,
                in1=lnw_sb, op0=OP.mult, op1=OP.mult)

        for b in range(NB):
            nc.sync.dma_start(out=pooled_d[b:b + 1, :],
                              in_=pooled_sb[0:1, b, :])

    nc.compile()
    return nc


def _build_nc_geglu(mm="bf16x2"):
    import concourse.bacc as bacc
    import concourse.mybir as mybir
    import concourse.tile as tile
    from contextlib import ExitStack

    f32 = mybir.dt.float32
    bf16 = mybir.dt.bfloat16
    comp = mm == "bf16x2"   # compensated bf16: hi/lo split of both operands
    mdt = f32 if mm == "fp32" else bf16
    NIN = 2 if comp else 1
    AF = mybir.ActivationFunctionType

    nc = bacc.Bacc("TRN2", target_bir_lowering=False, debug=False,
                   enable_asserts=False, num_devices=NCORES)

    pT_d = nc.dram_tensor("pT", [P, NIN, 8, B], mdt, kind="ExternalInput").ap()
    w_d = nc.dram_tensor("w", [NIN, 8, P, 2 * COLS], mdt, kind="ExternalInput").ap()
    bias_d = nc.dram_tensor("bias", [1, 2 * COLS], f32, kind="ExternalInput").ap()
    out_d = nc.dram_tensor("out", [B, COLS], f32, kind="ExternalOutput").ap()

    with tile.TileContext(nc) as tc, ExitStack() as ctx:
        singles = ctx.enter_context(tc.tile_pool(name="singles", bufs=1))
        tailp = ctx.enter_context(tc.tile_pool(name="tail", bufs=2))
        psum_pool = ctx.enter_context(
            tc.tile_pool(name="pspool", bufs=1, space="PSUM")
        )

        pT_sb = singles.tile([P, NIN, 8, B], mdt)
        nc.sync.dma_start(out=pT_sb, in_=pT_d)
        w_sb = singles.tile([P, NIN, 8, 2 * COLS], mdt)
        for n in range(NIN):
            for k in range(8):
                nc.sync.dma_start(out=w_sb[:, n, k], in_=w_d[n, k])
        bias_bc = singles.tile([B, 2 * COLS], f32)
        nc.sync.dma_start(out=bias_bc, in_=bias_d.to_broadcast([B, 2 * COLS]))

        terms = [(0, 0)] if not comp else [(0, 0), (1, 0), (0, 1)]
        hps = psum_pool.tile([B, 2 * COLS], f32, tag="acc")
        for ti, (pn, wn) in enumerate(terms):
            for k in range(8):
                for h in range(2):
                    nc.tensor.matmul(
                        hps[:, h * COLS:(h + 1) * COLS],
                        lhsT=pT_sb[:, pn, k, :],
                        rhs=w_sb[:, wn, k, h * COLS:(h + 1) * COLS],
                        start=(ti == 0 and k == 0),
                        stop=(ti == len(terms) - 1 and k == 7),
                    )
        hv = tailp.tile([B, COLS], f32, tag="hv")
        nc.vector.tensor_add(hv, hps[:, 0:COLS], bias_bc[:, 0:COLS])
        hg = tailp.tile([B, COLS], f32, tag="hg")
        nc.vector.tensor_add(hg, hps[:, COLS:2 * COLS], bias_bc[:, COLS:2 * COLS])
        gg = tailp.tile([B, COLS], f32, tag="gg")
        nc.scalar.activation(out=gg, in_=hg, func=AF.Gelu)
        outt = tailp.tile([B, COLS], f32, tag="outt")
        nc.vector.tensor_mul(outt, hv, gg)
        nc.sync.dma_start(out=out_d, in_=outt)

    nc.compile()
    return nc


def _pool_in_maps(x, ln_w, att_w):
    a = (ln_w * att_w[:, 0]).astype(np.float32).reshape(1, D)
    lnw = ln_w.astype(np.float32).reshape(1, D)
    xc = np.ascontiguousarray(x.astype(np.float32))
    cst = np.array([[1.0, EPS]], dtype=np.float32)
    return [
        {"x": xc[r * NB:(r + 1) * NB], "a": a, "lnw": lnw, "cst": cst}
        for r in range(NCORES)
    ]


def _split_hi_lo(arr, comp):
    import ml_dtypes
    if not comp:
        return arr.astype(ml_dtypes.bfloat16)[None]
    hi = arr.astype(ml_dtypes.bfloat16)
    lo = (arr - hi.astype(np.float32)).astype(ml_dtypes.bfloat16)
    return np.stack([hi, lo])


def _geglu_in_maps(pooled_full, geglu_w, geglu_b, mm="bf16x2"):
    comp = mm == "bf16x2"
    NIN = 2 if comp else 1
    if mm == "fp32":
        def conv(a):
            return a.astype(np.float32)[None]
    else:
        def conv(a):
            return _split_hi_lo(a, comp)
    pTn = np.ascontiguousarray(
        conv(np.ascontiguousarray(pooled_full.T))
    ).reshape(NIN, 8, P, B)
    pT = np.ascontiguousarray(np.transpose(pTn, (2, 0, 1, 3)))
    maps = []
    for r in range(NCORES):
        vs = slice(r * COLS, (r + 1) * COLS)
        gs = slice(OUT + r * COLS, OUT + (r + 1) * COLS)
        wcat = np.ascontiguousarray(
            np.concatenate([geglu_w[:, vs], geglu_w[:, gs]], axis=1)
        )
        wr = np.ascontiguousarray(conv(wcat)).reshape(NIN, 8, P, 2 * COLS)
        br = np.ascontiguousarray(
            np.concatenate([geglu_b[vs], geglu_b[gs]])
        ).reshape(1, 2 * COLS)
        maps.append({"pT": pT, "w": wr, "bias": br})
    return maps


LAST_RESULTS = None


def kernel(x, ln_w, att_w, att_b, geglu_w, geglu_b):
    global LAST_RESULTS
    from concourse.bass_utils import run_bass_kernel_spmd

    x = np.asarray(x, dtype=np.float32)
    ln_w = np.asarray(ln_w, dtype=np.float32)
    att_w = np.asarray(att_w, dtype=np.float32)
    geglu_w = np.asarray(geglu_w, dtype=np.float32)
    geglu_b = np.asarray(geglu_b, dtype=np.float32)
    # att_b is mathematically irrelevant (softmax shift-invariance)

    mm = os.environ.get("KERNEL_MM", "smaj")
    gg = os.environ.get("KERNEL_GG", "bf16")
    trace = os.environ.get("KERNEL_TRACE", "0") == "1"

    # Phase A: pooling
    if mm == "smaj":
        if ("A", mm) not in _cache:
            _cache[("A", mm)] = _build_nc_pool_smaj()
        res_a = run_bass_kernel_spmd(
            _cache[("A", mm)], _pool_smaj_in_maps(x, ln_w, att_w),
            core_ids=list(range(NCORES)), trace=trace,
        )
        pooled_full = _pool_smaj_unshard(res_a.results, ln_w)
    elif mm == "dmaj":
        if ("A", mm) not in _cache:
            _cache[("A", mm)] = _build_nc_pool_dmaj()
        res_a = run_bass_kernel_spmd(
            _cache[("A", mm)], _pool_dmaj_in_maps(x, ln_w, att_w),
            core_ids=list(range(NCORES)), trace=trace,
        )
        pooled_full = _pool_dmaj_unshard(res_a.results, ln_w)
    else:
        if ("A", mm) not in _cache:
            _cache[("A", mm)] = _build_nc_pool_classic()
        res_a = run_bass_kernel_spmd(
            _cache[("A", mm)], _pool_in_maps(x, ln_w, att_w),
            core_ids=list(range(NCORES)), trace=trace,
        )
        pooled_full = np.concatenate(
            [res_a.results[r]["pooled"] for r in range(NCORES)], axis=0
        )

    # Phase B: GeGLU readout
    if gg == "bf16":
        with_bias = bool(np.any(geglu_b))
        key = ("B", gg, with_bias)
        if key not in _cache:
            _cache[key] = _build_nc_geglu_bf16(with_bias)
        res_b = run_bass_kernel_spmd(
            _cache[key],
            _geglu_bf16_in_maps(pooled_full, geglu_w, geglu_b, with_bias),
            core_ids=list(range(NCORES)), trace=trace,
        )
    else:
        key = ("B", gg)
        if key not in _cache:
            _cache[key] = _build_nc_geglu(mm=gg)
        res_b = run_bass_kernel_spmd(
            _cache[key], _geglu_in_maps(pooled_full, geglu_w, geglu_b, mm=gg),
            core_ids=list(range(NCORES)), trace=trace,
        )
    LAST_RESULTS = (res_a, res_b)
    out = np.concatenate(
        [res_b.results[r]["out"] for r in range(NCORES)], axis=1
    )
    return out.astype(np.float32)
